# revision 1
# baseline (speedup 1.0000x reference)
"""Trainium2 Bass kernel for the DIN sparse-attention module.

Algorithm (per batch b):
  The reference computes din = [q, f, q-f, q*f] @ w1 per (q, f) pair.
  With W_A = w1[0:128]+w1[256:384], W_C = w1[128:256]-w1[256:384],
  W_P = w1[384:512], layer 1 collapses to
      z1[k, (q,f)] = W_P.T @ (F_T * Q_T[:,q]) + W_C.T @ F_T + W_A.T @ Q_T[:,q] + b1
  (the broadcast terms are computed as matmuls with step-0 access patterns).
  Layers 2/3 chain in [feature, (q,f)] layout.  Layer 3 uses an augmented
  contraction: 4 indicator rows select the diagonal q-block per output
  partition (others get -1e5 -> exp -> 0), so the scores land pre-spread
  across PSUM partitions 32*b_l + 4g + qq with no partition shuffle.
  Final output: out[q] = sum_f softmax(s)[q,f] * G[q,f],  G = Q @ F.T.

Sharding: pure data parallel, 8 batches per core across 8 cores.
"""

import numpy as np

import concourse.bass as bass
import concourse.bacc as bacc
import concourse.tile as tile
from concourse import mybir
from concourse.bass_utils import run_bass_kernel_spmd
from concourse.masks import make_identity

B, Q, F, D = 64, 32, 128, 128
N_CORES = 8
BPC = B // N_CORES          # batches per core
GQ = 4                      # queries per group
N_GROUPS = Q // GQ          # 8 groups per batch
PACK = 4                    # batches packed per tail pass
BIG = 1.0e5

f32 = mybir.dt.float32
bf16 = mybir.dt.bfloat16
i32 = mybir.dt.int32
AF = mybir.ActivationFunctionType
ALU = mybir.AluOpType


def _bcast(ap: bass.AP, reps: int, inner: int = 1) -> bass.AP:
    """Insert a step-0 broadcast dim before the last `inner` free dims."""
    dims = [list(d) for d in ap.ap]
    pos = len(dims) - inner
    new = dims[:pos] + [[0, reps]] + dims[pos:]
    return bass.AP(tensor=ap.tensor, offset=ap.offset, ap=new)


def build_program(reps: int = 1, debug_stage: int = 3):
    """debug_stage: 3=full, 2=no e-collect DMAs, 1=also no exp, 0=also no L3."""
    nc = bacc.Bacc("TRN2", target_bir_lowering=False, debug=False)

    query_t = nc.dram_tensor("query", [BPC, Q, D], f32, kind="ExternalInput")
    facts_t = nc.dram_tensor("facts", [BPC, F, D], f32, kind="ExternalInput")
    mask_t = nc.dram_tensor("mask", [BPC, F], i32, kind="ExternalInput")
    # host-preprocessed weights: wcat = [W_A | W_C | W_P] (128 x 240),
    # lhst3 = augmented layer-3 lhsT (68 x 128), indpat = indicator rows,
    # mu_t = transpose of M_u = W_P (W_P^T W_P)^-1 W_A^T (A-term fold).
    wcat_t = nc.dram_tensor("wcat", [D, 240], f32, kind="ExternalInput")
    mu_t = nc.dram_tensor("mu_t", [D, D], f32, kind="ExternalInput")
    b1_t = nc.dram_tensor("b1", [80], f32, kind="ExternalInput")
    w2_t = nc.dram_tensor("w2", [80, 40], f32, kind="ExternalInput")
    b2_t = nc.dram_tensor("b2", [40], f32, kind="ExternalInput")
    lhst3_t = nc.dram_tensor("lhst3", [68, 128], f32, kind="ExternalInput")
    indpat_t = nc.dram_tensor("indpat", [GQ, GQ * F], f32, kind="ExternalInput")
    out_t = nc.dram_tensor("out", [BPC, Q], f32, kind="ExternalOutput")

    with tile.TileContext(nc) as tc:
        with (
            tc.tile_pool(name="consts", bufs=1) as consts,
            tc.tile_pool(name="batch", bufs=2) as batch_pool,
            tc.tile_pool(name="grp", bufs=3) as grp_pool,
            tc.tile_pool(name="packp", bufs=2) as pack_pool,
            tc.tile_pool(name="ps1", bufs=2, space="PSUM") as ps1_pool,
            tc.tile_pool(name="ps2", bufs=2, space="PSUM") as ps2_pool,
            tc.tile_pool(name="ps3", bufs=2, space="PSUM") as ps3_pool,
            tc.tile_pool(name="psT", bufs=1, space="PSUM") as psT_pool,
            tc.tile_pool(name="psG", bufs=1, space="PSUM") as psG_pool,
        ):
            # ---------------- constants / weights ----------------
            identity = consts.tile([128, 128], f32)
            make_identity(nc, identity)

            wcat_sb = consts.tile([D, 240], f32)
            nc.sync.dma_start(out=wcat_sb, in_=wcat_t.ap())
            wcat_bf = consts.tile([D, 240], bf16)
            nc.vector.tensor_copy(wcat_bf, wcat_sb)
            W_C = wcat_bf[:, 80:160]
            W_P = wcat_bf[:, 160:240]

            mu_f = consts.tile([D, D], f32)
            nc.sync.dma_start(out=mu_f, in_=mu_t.ap())
            mu_bf = consts.tile([D, D], bf16)
            nc.vector.tensor_copy(mu_bf, mu_f)

            w2_sb = consts.tile([80, 40], f32)
            nc.sync.dma_start(out=w2_sb, in_=w2_t.ap())
            w2_bf = consts.tile([80, 40], bf16)
            nc.vector.tensor_copy(w2_bf, w2_sb)

            b1_sb = consts.tile([80, 1], f32)
            nc.sync.dma_start(
                out=b1_sb, in_=bass.AP(tensor=b1_t, offset=0, ap=[[1, 80], [1, 1]])
            )
            b2_sb = consts.tile([40, 1], f32)
            nc.sync.dma_start(
                out=b2_sb, in_=bass.AP(tensor=b2_t, offset=0, ap=[[1, 40], [1, 1]])
            )

            # lhsT for the augmented layer-3 matmul (host-built):
            #   rows 0:40 = w3 bcast, 40:64 = 0, 64:68 = diag-block penalties
            lhst3_f = consts.tile([68, 128], f32)
            nc.sync.dma_start(out=lhst3_f, in_=lhst3_t.ap())
            lhsT_aug = consts.tile([68, 128], bf16)
            nc.vector.tensor_copy(lhsT_aug, lhst3_f)

            indp_f = consts.tile([GQ, GQ * F], f32)
            nc.sync.dma_start(out=indp_f, in_=indpat_t.ap())

            # persistent rhs tiles for layer 3: rows 0:40 = h2 (rewritten per
            # group), rows 40:64 = 0, rows 64:68 = q-block indicator pattern
            # (1 on the diagonal 128-block, 0 elsewhere).
            h2_tiles = []
            for name in range(2):
                h2x = consts.tile([68, GQ * F], bf16, tag=f"h2x{name}")
                nc.vector.memset(h2x[32:64, :], 0.0)
                nc.vector.tensor_copy(h2x[64:68, :], indp_f)
                h2_tiles.append(h2x)

            # ---------------- main loop ----------------
            import os as _os

            n_packs = int(_os.environ.get("BASS_PACKS", str(BPC // PACK)))
            n_batches = int(_os.environ.get("BASS_BATCHES", str(PACK)))
            n_groups = int(_os.environ.get("BASS_GROUPS", str(N_GROUPS)))
            ss = int(_os.environ.get("BASS_SUBSTAGE", "99"))
            for _rep in range(reps):
              for pack in range(n_packs):
                G_ps = psG_pool.tile([128, F], f32)
                e_pack = pack_pool.tile([128, GQ, F], bf16)
                if debug_stage < 3:
                    nc.vector.memset(e_pack, 1.0)
                mask_i = pack_pool.tile([128, F], i32)

                for b_l in range(n_batches):
                    b = pack * PACK + b_l

                    # load + transpose query/facts
                    F_sb = batch_pool.tile([F, D], f32)
                    Q_sb = batch_pool.tile([Q, D], f32)
                    nc.sync.dma_start(out=F_sb, in_=facts_t.ap()[b])
                    nc.sync.dma_start(out=Q_sb, in_=query_t.ap()[b])

                    T_ps = psT_pool.tile([128, F], f32, tag="tps")
                    nc.tensor.transpose(T_ps, F_sb, identity)
                    F_Tb = batch_pool.tile([D, F], bf16)
                    nc.vector.tensor_copy(F_Tb, T_ps)

                    T2_ps = psT_pool.tile([128, Q], f32, tag="tps")
                    nc.tensor.transpose(T2_ps, Q_sb, identity[0:Q, 0:Q])
                    Q_T = batch_pool.tile([D, Q], f32)
                    Q_Tb = batch_pool.tile([D, Q], bf16)
                    nc.vector.tensor_copy(Q_T, T2_ps)
                    nc.vector.tensor_copy(Q_Tb, T2_ps)

                    # U = M_u @ Q_T reconstructs the A-term inside tmp4:
                    # W_P^T @ (F_T*Q_T[:,q] + U[:,q]) = pairwise + A[q] bcast
                    U_ps = psT_pool.tile([128, Q], f32, tag="tps")
                    nc.tensor.matmul(U_ps, mu_bf, Q_Tb, start=True, stop=True)
                    U_sb = batch_pool.tile([D, Q], f32)
                    nc.vector.tensor_copy(U_sb, U_ps)

                    if ss < 2:
                        continue
                    # G[q, f] = <query_q, facts_f>, col-tiled into partition
                    # strip 32*b_l of the pack-wide PSUM bank
                    nc.tensor.matmul(
                        G_ps[32 * b_l : 32 * b_l + 32, :],
                        Q_Tb,
                        F_Tb,
                        start=True,
                        stop=True,
                        tile_position=(0, 32 * b_l),
                    )

                    # mask rows for this batch (broadcast over the 32 queries)
                    nc.sync.dma_start(
                        out=mask_i[32 * b_l : 32 * b_l + 32, :],
                        in_=bass.AP(
                            tensor=mask_t, offset=b * F, ap=[[0, Q], [1, F]]
                        ),
                    )

                    for g in range(n_groups):
                        q0 = GQ * g
                        r0 = 32 * b_l + q0

                        if ss < 3:
                            continue
                        tmp4 = grp_pool.tile([D, GQ, F], bf16)
                        for qq in range(GQ):
                            nc.vector.tensor_scalar(
                                tmp4[:, qq],
                                F_Tb,
                                Q_T[:, q0 + qq : q0 + qq + 1],
                                U_sb[:, q0 + qq : q0 + qq + 1],
                                op0=ALU.mult,
                                op1=ALU.add,
                            )

                        ps1 = ps1_pool.tile([80, GQ * F], f32)
                        nc.tensor.matmul(
                            ps1,
                            W_P,
                            tmp4.rearrange("d g f -> d (g f)"),
                            start=True,
                            stop=False,
                        )
                        nc.tensor.matmul(
                            ps1, W_C, _bcast(F_Tb, GQ), start=False, stop=True
                        )

                        h1 = grp_pool.tile([80, GQ * F], bf16)
                        nc.scalar.activation(h1, ps1, AF.Sigmoid, bias=b1_sb)

                        if ss < 4:
                            continue
                        ps2 = ps2_pool.tile([40, GQ * F], f32)
                        nc.tensor.matmul(ps2, w2_bf, h1, start=True, stop=True)

                        if debug_stage < 1:
                            junk = grp_pool.tile([40, GQ * F], bf16, tag="junk")
                            nc.scalar.activation(
                                junk, ps2, AF.Sigmoid, bias=b2_sb
                            )
                            continue

                        h2x = h2_tiles[g % 2]
                        nc.scalar.activation(
                            h2x[0:40, :], ps2, AF.Sigmoid, bias=b2_sb
                        )

                        ps3 = ps3_pool.tile([128, GQ * F], f32)
                        nc.tensor.matmul(ps3, lhsT_aug, h2x, start=True, stop=True)

                        if debug_stage < 2:
                            continue

                        # exp the aligned 32-row strip (PSUM reads must start
                        # 32-aligned); DMA extracts the 4 valid rows into the
                        # unaligned pack position.
                        e_s = grp_pool.tile([32, GQ * F], bf16)
                        nc.scalar.activation(
                            e_s, ps3[32 * b_l : 32 * b_l + 32, :], AF.Exp
                        )
                        if debug_stage < 3:
                            continue
                        nc.sync.dma_start(
                            out=e_pack[r0 : r0 + GQ, :].rearrange(
                                "p g f -> p (g f)"
                            ),
                            in_=e_s[q0 : q0 + GQ, :],
                        )

                # ---------------- pack tail ----------------
                if ss < 5:
                    outcol0 = pack_pool.tile([128, 1], f32, tag="outcol")
                    nc.vector.memset(outcol0, 1.0)
                    nc.sync.dma_start(
                        out=bass.AP(
                            tensor=out_t, offset=128 * pack, ap=[[1, 128], [1, 1]]
                        ),
                        in_=outcol0,
                    )
                    continue
                mask_bf = pack_pool.tile([128, F], bf16)
                nc.vector.tensor_copy(mask_bf, mask_i)
                nc.vector.tensor_mul(e_pack, e_pack, _bcast(mask_bf, GQ))
                esum = pack_pool.tile([128, 1], f32)
                nc.vector.tensor_reduce(
                    esum, e_pack, axis=mybir.AxisListType.XY, op=ALU.add
                )

                G_bf = pack_pool.tile([128, F], bf16)
                nc.vector.tensor_copy(G_bf, G_ps)
                eg = pack_pool.tile([128, GQ, F], bf16)
                wsum = pack_pool.tile([128, 1], f32)
                # (tensor_tensor_reduce hangs this HW build -> mul + reduce)
                nc.vector.tensor_mul(eg, e_pack, _bcast(G_bf, GQ))
                nc.vector.tensor_reduce(
                    wsum, eg, axis=mybir.AxisListType.XY, op=ALU.add
                )

                rsum = pack_pool.tile([128, 1], f32)
                nc.vector.reciprocal(rsum, esum)
                outcol = pack_pool.tile([128, 1], f32)
                nc.vector.tensor_mul(outcol, wsum, rsum)

                nc.sync.dma_start(
                    out=bass.AP(
                        tensor=out_t, offset=128 * pack, ap=[[1, 128], [1, 1]]
                    ),
                    in_=outcol,
                )

    nc.compile()
    return nc


_CACHED = {}


def _get_program(reps: int = 1):
    import os

    stage = int(os.environ.get("BASS_DEBUG_STAGE", "3"))
    key = (reps, stage)
    if key not in _CACHED:
        _CACHED[key] = build_program(reps, debug_stage=stage)
    return _CACHED[key]


def _make_in_maps(inputs):
    query = np.ascontiguousarray(np.asarray(inputs["query"], np.float32))
    facts = np.ascontiguousarray(np.asarray(inputs["facts"], np.float32))
    mask = np.ascontiguousarray(np.asarray(inputs["mask"], np.int32))
    w1 = np.asarray(inputs["w1"], np.float32)
    b1 = np.ascontiguousarray(np.asarray(inputs["b1"], np.float32))
    w2 = np.ascontiguousarray(np.asarray(inputs["w2"], np.float32))
    b2 = np.ascontiguousarray(np.asarray(inputs["b2"], np.float32))
    w3 = np.asarray(inputs["w3"], np.float32)

    # weight preprocessing (tiny, host-side): layer-1 decomposition
    W_A = w1[0:128] + w1[256:384]
    W_C = w1[128:256] - w1[256:384]
    W_P = w1[384:512]
    wcat = np.ascontiguousarray(np.concatenate([W_A, W_C, W_P], axis=1))

    # M_u: minimum-norm solution of W_P^T u = W_A^T q for all q, i.e.
    # u = M_u q with M_u = W_P (W_P^T W_P)^-1 W_A^T.  Folds the A-term into
    # the tensor_scalar that builds tmp4.
    gram = (W_P.T @ W_P).astype(np.float64)
    M_u = (W_P @ np.linalg.solve(gram, W_A.T.astype(np.float64))).astype(np.float32)
    mu_t_host = np.ascontiguousarray(M_u.T)

    # augmented layer-3 lhsT: w3 replicated over the 128 output partitions,
    # plus per-q-block diagonal penalties
    lhst3 = np.zeros((68, 128), np.float32)
    lhst3[0:40, :] = w3[:, 0:1]
    m = np.arange(128)
    for j in range(GQ):
        lhst3[64 + j, :] = np.where(m % GQ == j, 0.0, -BIG)

    indpat = np.zeros((GQ, GQ * F), np.float32)
    for j in range(GQ):
        indpat[j, j * F : (j + 1) * F] = 1.0

    in_maps = []
    for c in range(N_CORES):
        sl = slice(c * BPC, (c + 1) * BPC)
        in_maps.append(
            {
                "query": np.ascontiguousarray(query[sl]),
                "facts": np.ascontiguousarray(facts[sl]),
                "mask": np.ascontiguousarray(mask[sl]),
                "wcat": wcat,
                "mu_t": mu_t_host,
                "b1": b1,
                "w2": w2,
                "b2": b2,
                "lhst3": lhst3,
                "indpat": indpat,
            }
        )
    return in_maps


def run_traced(inputs, trace=False, reps=1):
    """Run on all 8 NeuronCores; returns (out [64,32] f32, exec_time_ns|None)."""
    nc = _get_program(reps)
    res = run_bass_kernel_spmd(
        nc, _make_in_maps(inputs), core_ids=list(range(N_CORES)), trace=trace
    )
    out = np.concatenate([res.results[c]["out"] for c in range(N_CORES)], axis=0)
    return out.astype(np.float32), res.exec_time_ns


def kernel(**inputs) -> np.ndarray:
    out, _ = run_traced(inputs, trace=False)
    return out

